# revision 78
# baseline (speedup 1.0000x reference)
"""Trainium2 Bass kernel for fused sparse-attention block (nn_Attention_790273982568).

Full (unsharded) inputs in, full output out. Internally: tensor-parallel over
heads across 8 NeuronCores — each core owns 4 Q heads + 1 KV head (wqkv rows)
and 512 output columns of wo (rows of wo), with span-granular on-device
AllGathers of the attention outputs (overlapped with attention) before the
output projection.

The QKV projection and the output projection both run as 3-term fp8
DoubleRow matmuls (operands split exactly into e4m3 hi+lo planes — x/wqkv/wo
on the host, the attention outputs on the idle gpsimd engine; the dropped
lo@lo term is ~2^-8 relative), contracting 256 elements per instruction at
0.5 cycles/row — 0.75x the PE time of one bf16 pass. The AllGather carries
the fp8 hi+lo planes token-interleaved (same bytes as bf16). LayerNorm +
RoPE are batched across the 5 heads per token tile with stride-0 broadcast
APs, and the identity LN weight/bias application is elided (general fallback
program otherwise).
"""

import os
import sys

import numpy as np

for _p in ("/opt/trn_rl_repo", "/root/.axon_site/_ro/trn_rl_repo"):
    if _p not in sys.path and os.path.isdir(_p):
        sys.path.append(_p)

import ml_dtypes  # noqa: E402

import bass_rust as _bass_rust  # noqa: E402
import concourse.bass as bass  # noqa: E402
from concourse import bacc  # noqa: E402
import concourse.mybir as mybir  # noqa: E402
import concourse.tile as tile  # noqa: E402
from concourse.bass import ds, ts  # noqa: E402
from concourse.bass_utils import run_bass_kernel_spmd  # noqa: E402

# Problem shapes (hardcoded per spec)
T = 2048
DIM = 4096
HD = 128
NH = 32
NKV = 8
NCORES = 8
QH = NH // NCORES          # 4 q heads per core
FEAT = (QH + 2) * HD       # 768 qkv features per core
OUTC = DIM // NCORES       # 512 output columns per core
P = 128
NT = T // P                # 16 token tiles
KC = DIM // P              # 32 contraction chunks
QSPAN = 512
NQS = T // QSPAN           # 4 q spans
HALF = HD // 2
EPS = 1e-5
THETA = 10000.0
SCALE = 1.0 / float(np.sqrt(HD))

BF16 = mybir.dt.bfloat16
F32 = mybir.dt.float32
FP8 = mybir.dt.float8e4
AX = mybir.AxisListType
ALU = mybir.AluOpType
ACTF = mybir.ActivationFunctionType
DR = mybir.MatmulPerfMode.DoubleRow

# wqkv rows are scaled by WSCALE on the host before the exact bf16 ->
# fp8(hi)+fp8(lo) split so the lo plane stays clear of the e4m3 subnormal
# floor; the QKV psum is descaled by 1/WSCALE (exact power of two).
WSCALE = 64.0
# attention outputs are scaled by AOS before their fp8 hi/lo split; wo is
# scaled by WSCALE before its split; the phase-3 evac descales by both.
AOS = 16.0

_PROGRAM_CACHE = {}


def _build_body(nc, aps):
    woTh, woTl = aps["woTh"], aps["woTl"]
    ropeP = aps["ropeP"]
    lnwb = aps.get("lnwb")
    masks = aps["masks"]
    ident = aps["ident"]
    ag_in = aps["ag_in"]
    ag_out = aps["ag_out"]       # [QH, NCORES*P, T]
    outT = aps["outT"]
    tc = aps["tc"]

    with (
        tc.tile_pool(name="consts", bufs=1) as consts,
        tc.tile_pool(name="prq", bufs=3) as prq_pool,
        tc.tile_pool(name="psumTr", bufs=2, space="PSUM") as psumTr,
    ):
        ident_sb = consts.tile([P, P], BF16, tag="ident")
        nc.sync.dma_start(ident_sb[:], ident[:, :])
        masks_sb = consts.tile([P, 4, QSPAN], BF16, tag="masks")

        # persistent activation strips
        qkT = consts.tile([P, QH + 1, T], BF16, tag="qkT")       # [hd, head, tok]
        vaug = consts.tile([P, NT, HD + 1], BF16, tag="vaug")    # [ktok%, ktile, hd+1]
        # span (0,0)'s attention weights, precomputed at the end of phase 1
        attn00 = consts.tile([P, 4, QSPAN], BF16, tag="attn00")
        nc.vector.memset(vaug[:, :, HD : HD + 1], 1.0)

        # deferred phase-1 transposes: the last token tiles' LN/rope chains
        # are still draining on DVE when phase 2 starts; their transposes are
        # emitted into the phase-2 PE stream right before the first j=3 span
        # (the first consumer) so they don't head-of-line-block early spans.
        rq_pend = []

        def drain_rq():
            pt, prq = rq_pend.pop(0)
            for h in range(5):
                ptr = psumTr.tile([P, P], BF16, tag="ptr")
                nc.tensor.transpose(ptr[:], prq[:, ds(h * HD, HD)], ident_sb[:])
                # late tiles' evacs go on DVE: on ACT they would queue behind
                # the last tiles' LN stats (delaying the first exps, which the
                # score-psum recycle waits on) or behind the early spans' exps
                # (delaying the j=3 strips)
                if pt >= NT - 4:
                    nc.vector.tensor_copy(qkT[:, h, ts(pt, P)], ptr[:])
                else:
                    nc.scalar.copy(qkT[:, h, ts(pt, P)], ptr[:])

        # ---------------- Phase 1: QKV projection + LN + RoPE ----------------
        # QKV runs as 3-term fp8 DoubleRow matmuls on host-split operands:
        # x = xh + xl and w = wh + wl (exact bf16 -> e4m3 hi/lo splits), with
        # x@w ~= xh@wh + xl@wh + xh@wl (the dropped xl@wl term is ~2^-8
        # relative). DoubleRow contracts two 128-chunks per instruction at
        # 0.5 cycles/row, so the three terms cost 0.75x one bf16 pass.
        with (
            tc.tile_pool(name="wq", bufs=1) as wq_pool,
            tc.tile_pool(name="p1", bufs=3) as p1,
            tc.tile_pool(name="px", bufs=2) as px,
            tc.tile_pool(name="p1s", bufs=4) as p1s,
            tc.tile_pool(name="psum1", bufs=3, space="PSUM") as psum1,
        ):
            apply_lnwb = aps.get("apply_lnwb", False)

            def load_xt(t, eng=None):
                # 4-tile (512-token) spans keep the contiguous run at 512B:
                # smaller fp8 runs pay a 2x DMA read-modify-write penalty
                th_ = px.tile([P, KC, 4 * P], FP8, tag="xth", name=f"xth_{t}")
                tl_ = px.tile([P, KC, 4 * P], FP8, tag="xtl", name=f"xtl_{t}")
                for g in range(KC // 8):
                    e = eng or (nc.sync if g % 2 == 0 else nc.scalar)
                    for src, dst in ((xTh, th_), (xTl, tl_)):
                        e.dma_start(
                            dst[:, ds(8 * g, 8), :],
                            src[ds(8 * g * P, 8 * P), ds(t * P, 4 * P)].rearrange(
                                "(k p) c -> p k c", p=P
                            ),
                        )
                return th_, tl_

            xTh, xTl = aps["xTh"], aps["xTl"]
            wqkvTh, wqkvTl = aps["wqkvTh"], aps["wqkvTl"]

            # stripe x/weight chunks across both HWDGE rings, k-interleaved,
            # so matmul k can start as soon as chunk k has landed; first pack
            # at single-chunk granularity to cut the cold-start stall
            xt0h = px.tile([P, KC, 4 * P], FP8, tag="xth", name="xth_0")
            xt0l = px.tile([P, KC, 4 * P], FP8, tag="xtl", name="xtl_0")
            wh_sb = wq_pool.tile([P, KC, FEAT], FP8, tag="wqkvTh")
            wl_sb = wq_pool.tile([P, KC, FEAT], FP8, tag="wqkvTl")
            # hi planes of the first chunk-pair first, one DMA each: the
            # first matmul needs exactly {wh,xh} x {chunk0,chunk1}, and every
            # extra DMA ahead of it costs 625ns on the HWDGE singleton
            nc.sync.dma_start(
                wh_sb[:, ds(0, 2), :],
                wqkvTh[ds(0, 2 * P), :].rearrange("(k p) f -> p k f", p=P),
            )
            nc.scalar.dma_start(
                xt0h[:, ds(0, 2), :],
                xTh[ds(0, 2 * P), ds(0, 4 * P)].rearrange(
                    "(k p) c -> p k c", p=P
                ),
            )
            nc.scalar.dma_start(
                xt0l[:, ds(0, 2), :],
                xTl[ds(0, 2 * P), ds(0, 4 * P)].rearrange(
                    "(k p) c -> p k c", p=P
                ),
            )
            nc.sync.dma_start(
                wl_sb[:, ds(0, 2), :],
                wqkvTl[ds(0, 2 * P), :].rearrange("(k p) f -> p k f", p=P),
            )
            for g in range(KC // 4):
                lo = max(4 * g, 2)
                n = 4 * (g + 1) - lo
                if n <= 0:
                    continue
                e0, e1 = (nc.sync, nc.scalar) if g % 2 == 0 else (nc.scalar, nc.sync)
                for src, dst in ((wqkvTh, wh_sb), (wqkvTl, wl_sb)):
                    e0.dma_start(
                        dst[:, ds(lo, n), :],
                        src[ds(lo * P, n * P), :].rearrange(
                            "(k p) f -> p k f", p=P
                        ),
                    )
                for src, dst in ((xTh, xt0h), (xTl, xt0l)):
                    e1.dma_start(
                        dst[:, ds(lo, n), :],
                        src[ds(lo * P, n * P), ds(0, 4 * P)].rearrange(
                            "(k p) c -> p k c", p=P
                        ),
                    )
            xt_cache = {0: (xt0h, xt0l)}
            # rope table in two halves and masks deferred: the early DMA
            # queue is tight (second x span must land by tile 4) and only
            # the first-half cos/sin rows are needed before then
            rope_sb = wq_pool.tile([P, NT, 2, HALF], F32, tag="rope")
            nc.sync.dma_start(rope_sb[:, 0 : NT // 2], ropeP[:, 0 : NT // 2])
            wb_sb = wq_pool.tile([P, 5, 2, HD], F32, tag="wb")
            if apply_lnwb:
                nc.sync.dma_start(wb_sb[:], lnwb[:, :, :, :])

            for t in range(NT):
                if t == NT - 1:
                    # precompute span (0,0): its exps sit ahead of the last
                    # tile's LN work in the ACT queue and phase 2 opens with
                    # attn(0,0) ready, so the pv(0,0) work overlaps the
                    # first score-psum recycle wait
                    for i00 in range(4):
                        ps00 = psumTr.tile([P, QSPAN], F32, tag="ptr")
                        nc.tensor.matmul(
                            ps00[:], qkT[:, QH, ts(i00, P)],
                            qkT[:, 0, ds(0, QSPAN)],
                            start=True, stop=True,
                        )
                        nc.scalar.activation(
                            attn00[:, i00, :], ps00[:], ACTF.Exp, scale=SCALE
                        )
                    for i00 in range(0, 4, 2):
                        nc.vector.tensor_mul(
                            attn00[:, i00 : i00 + 2, :],
                            attn00[:, i00 : i00 + 2, :],
                            masks_sb[:, i00 : i00 + 2, :],
                        )
                if t == 4:
                    # second halves of the const loads, after the hot window
                    nc.sync.dma_start(
                        rope_sb[:, NT // 2 :], ropeP[:, NT // 2 :]
                    )
                    nc.sync.dma_start(masks_sb[:], masks[:, :, :])
                if t % 4 == 0:
                    xth_, xtl_ = (
                        xt_cache.pop(t) if t in xt_cache else load_xt(t)
                    )
                sub = t % 4
                pq = psum1.tile([P, FEAT], F32, tag="pqkv")
                npair = KC // 2
                for jp in range(npair):
                    for ti, (lt, rt) in enumerate(
                        ((xth_, wh_sb), (xtl_, wh_sb), (xth_, wl_sb))
                    ):
                        st = jp == 0 and ti == 0
                        sp = jp == npair - 1 and ti == 2
                        lhsT = lt[:, ds(2 * jp, 2), ds(sub * P, P)]
                        nc.tensor.matmul(
                            pq[:, 0:512], lhsT, rt[:, ds(2 * jp, 2), 0:512],
                            start=st, stop=sp, perf_mode=DR,
                        )
                        nc.tensor.matmul(
                            pq[:, 512:FEAT], lhsT, rt[:, ds(2 * jp, 2), 512:FEAT],
                            start=st, stop=sp, perf_mode=DR,
                        )
                # psum-draining copies ride on ACT: the DVE queue backlog
                # otherwise delays the psum release that phase 2's first
                # scores (bank reuse) wait on; descale by 1/WSCALE in the copy
                nc.scalar.activation(
                    vaug[:, t, 0:HD], pq[:, 640:FEAT], ACTF.Copy,
                    scale=1.0 / WSCALE,
                )
                # q/k slices as bf16, descaled (match reference's bf16 xqkv)
                xq = p1.tile([P, 5 * HD], BF16, tag="xq")
                nc.scalar.activation(
                    xq[:], pq[:, 0 : 5 * HD], ACTF.Copy, scale=1.0 / WSCALE
                )
                xqv = xq.rearrange("p (h d) -> p h d", h=5)

                # -------- LayerNorm, batched over the 5 heads --------
                s1 = p1s.tile([P, 5, 1], F32, tag="s1")
                nc.vector.reduce_sum(s1[:], xqv, axis=AX.X)
                negmu = p1s.tile([P, 5], F32, tag="negmu")
                nc.vector.tensor_scalar_mul(negmu[:], s1[:, :, 0], -1.0 / HD)
                # one batched Square + a segmented DVE reduce instead of 5
                # per-head accum Squares: shorter ACT queue ahead of the
                # first attention exps at the phase boundary
                sqs = p1s.tile([P, 5 * HD], F32, tag="sqs")
                nc.scalar.activation(sqs[:], xq[:], ACTF.Square)
                ssq = p1s.tile([P, 5, 1], F32, tag="ssq")
                nc.vector.reduce_sum(
                    ssq[:], sqs.rearrange("p (h d) -> p h d", h=5), axis=AX.X
                )
                varg = p1s.tile([P, 5], F32, tag="varg")
                nc.vector.tensor_scalar(
                    varg[:], ssq[:, :, 0], 1.0 / HD, EPS, op0=ALU.mult, op1=ALU.add
                )
                mu2 = p1s.tile([P, 5], F32, tag="mu2")
                nc.vector.tensor_mul(mu2[:], negmu[:], negmu[:])
                nc.vector.tensor_sub(varg[:], varg[:], mu2[:])
                stdv = p1s.tile([P, 5], F32, tag="stdv")
                nc.scalar.activation(stdv[:], varg[:], ACTF.Sqrt)
                rstd = p1s.tile([P, 5], F32, tag="rstd")
                nc.vector.reciprocal(rstd[:], stdv[:])
                xn = p1.tile([P, 5 * HD], F32, tag="xn")
                xnv = xn.rearrange("p (h d) -> p h d", h=5)
                nc.vector.tensor_add(
                    xnv, xqv,
                    negmu[:, :, None].broadcast_to([P, 5, HD]),
                )
                nc.vector.tensor_mul(
                    xnv, xnv,
                    rstd[:, :, None].broadcast_to([P, 5, HD]),
                )
                if apply_lnwb:
                    nc.vector.tensor_mul(xnv, xnv, wb_sb[:, :, 0, :])
                    nc.vector.tensor_add(xnv, xnv, wb_sb[:, :, 1, :])

                # -------- RoPE, batched over the 5 heads --------
                xr = xn.rearrange("p (h f two) -> p h two f", h=5, two=2)
                cosb = rope_sb[:, t, 0, :][:, None, :].broadcast_to([P, 5, HALF])
                sinb = rope_sb[:, t, 1, :][:, None, :].broadcast_to([P, 5, HALF])
                ta = p1.tile([P, 5, HALF], F32, tag="ta")
                tb = p1.tile([P, 5, HALF], F32, tag="tb")
                rq = prq_pool.tile([P, 5 * HD], BF16, tag="rq", name=f"rq_{t}")
                rqr = rq.rearrange("p (h f two) -> p h two f", h=5, two=2)
                nc.vector.tensor_mul(ta[:], xr[:, :, 0, :], cosb)
                nc.vector.tensor_mul(tb[:], xr[:, :, 1, :], sinb)
                nc.vector.tensor_sub(rqr[:, :, 0, :], ta[:], tb[:])
                nc.vector.tensor_mul(ta[:], xr[:, :, 0, :], sinb)
                nc.vector.tensor_mul(tb[:], xr[:, :, 1, :], cosb)
                nc.vector.tensor_add(rqr[:, :, 1, :], ta[:], tb[:])
                rq_pend.append((t, rq))
                # transpose the previous token block (gives DVE a full block
                # of slack before PE needs the rope output); the final two
                # tiles stay pending into phase 2
                while len(rq_pend) > 2:
                    drain_rq()

        # ---------------- Phase 2: attention (+ per-head AllGather) ----------
        with (
            tc.tile_pool(name="w3", bufs=1) as w3,
            tc.tile_pool(name="p3", bufs=19) as p3,
            tc.tile_pool(name="p3o", bufs=4) as p3o,
            tc.tile_pool(name="paoT", bufs=3) as paoT,
        ):
            # prefetch wo weights (fp8 hi/lo planes) while attention runs
            woTh_sb = w3.tile([P, KC, OUTC], FP8, tag="woTh")
            woTl_sb = w3.tile([P, KC, OUTC], FP8, tag="woTl")
            for k2 in range(KC // 2):
                for srcw, dstw in ((woTh, woTh_sb), (woTl, woTl_sb)):
                    nc.sync.dma_start(
                        dstw[:, ds(2 * k2, 2), :],
                        srcw[ds(2 * k2 * P, 2 * P), :].rearrange(
                            "(two p) f -> p two f", p=P
                        ),
                    )

            # first-half ao pair-tiles [feat128, chunk-pair, hi/lo, tok]
            ao0 = [None] * (KC // 2)
            with (
                tc.tile_pool(name="p2", bufs=2) as p2,
                tc.tile_pool(name="p2s", bufs=4) as p2s,
                tc.tile_pool(name="paob", bufs=8) as paob,
                tc.tile_pool(name="psum_s", bufs=2, space="PSUM") as psum_s_pool,
                tc.tile_pool(name="psum_o", bufs=2, space="PSUM") as psum_o_pool,
            ):
                def emit_scores(h, j):
                    nkb = 4 * (j + 1)
                    attn = p2.tile([P, NT, QSPAN], BF16, tag="attn",
                                   name=f"attn_{h}_{j}")
                    for ip in range(nkb // 2):
                        i = 2 * ip
                        ps = psum_s_pool.tile([P, 2, QSPAN], F32, tag="ps")
                        for u in range(2):
                            nc.tensor.matmul(
                                ps[:, u, :],
                                qkT[:, QH, ts(i + u, P)],
                                qkT[:, h, ds(j * QSPAN, QSPAN)],
                                start=True, stop=True,
                            )
                        # one exp over both blocks (amortize ACT fixed cost)
                        _ei = nc.scalar.activation(
                            attn[:, i : i + 2, :], ps[:], ACTF.Exp, scale=SCALE
                        )
                        r = i - 4 * j
                        if r >= 0:
                            # diagonal pair: one masking mul over both blocks
                            nc.vector.tensor_mul(
                                attn[:, i : i + 2, :],
                                attn[:, i : i + 2, :],
                                masks_sb[:, r : r + 2, :],
                            )
                    return attn

                def emit_pv_mm(h, j, attn):
                    aobs = []
                    for q4 in range(4):
                        qb = 4 * j + q4
                        po = psum_o_pool.tile([P, HD + 1], F32, tag="po")
                        for i in range(qb + 1):
                            nc.tensor.matmul(
                                po[:],
                                attn[:, i, ts(q4, P)],
                                vaug[:, i, :],
                                start=(i == 0), stop=(i == qb),
                            )
                        recip = p2s.tile([P, 1], F32, tag="recip")
                        nc.vector.reciprocal(recip[:], po[:, HD : HD + 1])
                        aob = paob.tile([P, HD], BF16, tag="aob",
                                        name=f"aob_{h}_{qb}")
                        # scaled by AOS for the fp8 hi/lo split downstream
                        nc.vector.tensor_scalar(
                            aob[:], po[:, 0:HD], recip[:], AOS,
                            op0=ALU.mult, op1=ALU.mult,
                        )
                        aobs.append(aob)
                    return aobs

                def _fetch_ao0(h):
                    # prefetch this head's first-half ao pair-tiles for ph 3
                    for rp in range(NCORES // 2):
                        kp = h * (NCORES // 2) + rp
                        a = p3.tile([P, 2, T // 2, 2], FP8, tag="ao",
                                    name=f"ao_0_{kp}")
                        nc.sync.dma_start(
                            a.rearrange("p two t l -> p two (t l)"),
                            ag_out[h, ds(2 * rp * P, 2 * P), ds(0, T // 2), :]
                            .rearrange("(two p) t l -> p two (t l)", p=P),
                        )
                        ao0[kp] = a

                def emit_tr(h, j, aobs, aoTh):
                    for q4 in range(4):
                        qb = 4 * j + q4
                        pt2 = psumTr.tile([P, P], BF16, tag="ptr")
                        nc.tensor.transpose(pt2[:], aobs[q4][:], ident_sb[:])
                        nc.vector.tensor_copy(aoTh[:, ts(qb, P)], pt2[:])
                    # split the span into exact fp8 hi+lo planes on the idle
                    # gpsimd engine, then ship it as soon as it is ready so
                    # the gather traffic is spread across phase 2 instead of
                    # bursting at each head's end (DMA device is the phase-2/3
                    # boundary bottleneck)
                    span = ds(j * QSPAN, QSPAN)
                    aoF = aoFs[h]
                    nc.gpsimd.tensor_copy(aoF[:, span, 0], aoTh[:, span])
                    nc.gpsimd.tensor_sub(
                        aoF[:, span, 1], aoTh[:, span], aoF[:, span, 0]
                    )
                    nc.sync.dma_start(ag_in[ts(h, P), span, :], aoF[:, span, :])
                    if aps.get("no_collective"):
                        nc.sync.dma_start(
                            ag_out[h][:, span, :].rearrange(
                                "(r p) t l -> p r (t l)", r=NCORES
                            ),
                            aoF[:, None, span, :].broadcast_to(
                                [P, NCORES, QSPAN, 2]
                            ).rearrange("p r t l -> p r (t l)"),
                        )
                        if j == 1:
                            # first token half fully gathered -> fetch early
                            _fetch_ao0(h)
                    elif j == NQS - 1:
                        nc.gpsimd.collective_compute(
                            "AllGather",
                            ALU.bypass,
                            replica_groups=[list(range(NCORES))],
                            ins=[ag_in[ts(h, P), :]],
                            outs=[ag_out[h]],
                        )
                        # must come after the collective in program order
                        # (reads ag_out, which the collective writes)
                        _fetch_ao0(h)

                # software pipeline: scores(j) | pv(j-2) | transpose(j-3)
                from collections import deque

                pv_q = deque()   # (h, j, attn)
                tr_q = deque()   # (h, j, aobs, aoTh)
                aoThs = {}
                # h-major, but each head's last (j=3) span is delayed past the
                # next head's first span: the j=3 scores need the final token
                # tile's K strip, whose LN/rope/transpose chain is still
                # draining when phase 2 starts.
                spans = [
                    (0, 0), (0, 1), (0, 2),
                    (1, 0), (0, 3), (1, 1), (1, 2),
                    (2, 0), (1, 3), (2, 1), (2, 2),
                    (3, 0), (2, 3), (3, 1), (3, 2), (3, 3),
                ]

                def step_pv():
                    ph, pj, pattn = pv_q.popleft()
                    tr_q.append((ph, pj, emit_pv_mm(ph, pj, pattn), aoThs[ph]))

                def step_tr():
                    emit_tr(*tr_q.popleft())

                aoFs = {}
                aoThs[0] = paoT.tile([P, T], BF16, tag="aoTh", name="aoT_0")
                aoFs[0] = paoT.tile([P, T, 2], FP8, tag="aoF", name="aoF_0")
                pv_q.append((0, 0, attn00))
                for h, j in spans[1:]:
                    if j == 3 and rq_pend:
                        while rq_pend:
                            drain_rq()
                    if j == 0:
                        aoThs[h] = paoT.tile(
                            [P, T], BF16, tag="aoTh", name=f"aoT_{h}"
                        )
                        aoFs[h] = paoT.tile(
                            [P, T, 2], FP8, tag="aoF", name=f"aoF_{h}"
                        )
                    attn = emit_scores(h, j)
                    pv_q.append((h, j, attn))
                    if len(pv_q) > 1:
                        step_pv()
                    if len(tr_q) > 1:
                        step_tr()
                while pv_q:
                    step_pv()
                    while len(tr_q) > 1:
                        step_tr()
                while tr_q:
                    step_tr()

            # ---------------- Phase 3: output projection ----------------
            # 2-psum column sweeps: per (token-half, col-block) sweep all 32
            # contraction chunks with one stationary load per chunk, then
            # evacuate immediately (DVE+ACT split) while the next sweep runs.
            # Keeps the post-last-matmul tail to a single 2-psum drain.
            with tc.tile_pool(name="psum3", bufs=6, space="PSUM") as psum3:
                ao1 = [None] * (KC // 2)
                NKP = KC // 2
                for th in range(2):  # token halves
                    for cb in range(4):  # column blocks of 128
                        pos = [
                            psum3.tile([P, 512], F32, tag="po3",
                                       name=f"po3_{th}_{cb}_{i}")
                            for i in range(2)
                        ]
                        for kp in range(NKP):
                            if th == 0 and cb == 2 and kp < NKP // 2:
                                # prefetch second-token-half ao pair-tiles a
                                # full sweep ahead of their first use, on the
                                # ACT dma queue so they don't block out-writes
                                for qq in range(2):
                                    pkp = 2 * kp + qq
                                    ph, prp = divmod(pkp, NCORES // 2)
                                    a1 = p3.tile([P, 2, T // 2, 2], FP8,
                                                 tag="ao", name=f"ao_1_{pkp}")
                                    nc.scalar.dma_start(
                                        a1.rearrange("p two t l -> p two (t l)"),
                                        ag_out[ph, ds(2 * prp * P, 2 * P),
                                               ds(T // 2, T // 2), :]
                                        .rearrange(
                                            "(two p) t l -> p two (t l)", p=P
                                        ),
                                    )
                                    ao1[pkp] = a1
                            a = ao0[kp] if th == 0 else ao1[kp]
                            for s2 in range(2):
                                for ti in range(3):
                                    w_sb = woTh_sb if ti < 2 else woTl_sb
                                    pl = ti % 2  # hi, lo, hi
                                    nc.tensor.matmul(
                                        pos[s2][:],
                                        w_sb[:, ds(2 * kp, 2), ts(cb, P)],
                                        a[:, :, ds(s2 * 512, 512), pl],
                                        start=(kp == 0 and ti == 0),
                                        stop=(kp == NKP - 1 and ti == 2),
                                        perf_mode=DR,
                                    )
                        for s2 in range(2):
                            ob = p3o.tile(
                                [P, 512], BF16, tag="ob",
                                name=f"ob_{th}_{cb}_{s2}"
                            )
                            # split evacuation across DVE and ACT so the
                            # final drain isn't serial on one engine
                            if s2 == 0:
                                nc.vector.tensor_scalar_mul(
                                    ob[:], pos[s2][:], 1.0 / (AOS * WSCALE)
                                )
                            else:
                                nc.scalar.activation(
                                    ob[:], pos[s2][:], ACTF.Copy,
                                    scale=1.0 / (AOS * WSCALE),
                                )
                            (nc.sync if s2 == 0 else nc.scalar).dma_start(
                                outT[ts(cb, P), ds(th * (T // 2) + s2 * 512, 512)],
                                ob[:],
                            )


def _build_program(no_collective=False, reps=1, apply_lnwb=False):
    nc = bacc.Bacc(
        "TRN2",
        target_bir_lowering=False,
        debug=False,
        enable_asserts=True,
        num_devices=1 if no_collective else NCORES,
    )
    aps = {
        "xTh": nc.dram_tensor("xTh", [DIM, T], FP8, kind="ExternalInput").ap(),
        "xTl": nc.dram_tensor("xTl", [DIM, T], FP8, kind="ExternalInput").ap(),
        "wqkvTh": nc.dram_tensor(
            "wqkvTh", [DIM, FEAT], FP8, kind="ExternalInput"
        ).ap(),
        "wqkvTl": nc.dram_tensor(
            "wqkvTl", [DIM, FEAT], FP8, kind="ExternalInput"
        ).ap(),
        "woTh": nc.dram_tensor("woTh", [NH * HD, OUTC], FP8, kind="ExternalInput").ap(),
        "woTl": nc.dram_tensor("woTl", [NH * HD, OUTC], FP8, kind="ExternalInput").ap(),
        "ropeP": nc.dram_tensor(
            "ropeP", [P, NT, 2, HALF], F32, kind="ExternalInput"
        ).ap(),
        "masks": nc.dram_tensor("masks", [P, 4, QSPAN], BF16, kind="ExternalInput").ap(),
        "ident": nc.dram_tensor("ident", [P, P], BF16, kind="ExternalInput").ap(),
        "ag_in": nc.dram_tensor("ag_in", [QH * HD, T, 2], FP8).ap(),
        "ag_out": nc.dram_tensor(
            "ag_out", [QH, NCORES * P, T, 2], FP8, addr_space="Shared"
        ).ap(),
        "outT": nc.dram_tensor("outT", [OUTC, T], BF16, kind="ExternalOutput").ap(),
    }
    if apply_lnwb:
        aps["lnwb"] = nc.dram_tensor(
            "lnwb", [P, 5, 2, HD], F32, kind="ExternalInput"
        ).ap()
    aps["no_collective"] = no_collective
    aps["apply_lnwb"] = apply_lnwb
    with tile.TileContext(nc) as tc:
        aps["tc"] = tc
        for _rep in range(reps):
            _build_body(nc, aps)
    nc.compile()
    return nc


def get_program(apply_lnwb=False):
    key = ("nc", apply_lnwb)
    if key not in _PROGRAM_CACHE:
        _PROGRAM_CACHE[key] = _build_program(apply_lnwb=apply_lnwb)
    return _PROGRAM_CACHE[key]


def _rope_tables():
    """cos/sin tables computed exactly like the reference (jax fp32 on cpu)."""
    try:
        import jax

        cpu = jax.devices("cpu")[0]
        with jax.default_device(cpu):
            import jax.numpy as jnp

            inv_freq = 1.0 / (
                THETA ** (jnp.arange(HALF, dtype=jnp.float32) * 2.0 / HD)
            )
            pos = jnp.arange(T, dtype=jnp.float32)
            ang = pos[:, None] * inv_freq[None, :]
            cos = np.asarray(jnp.cos(ang), dtype=np.float32)
            sin = np.asarray(jnp.sin(ang), dtype=np.float32)
    except Exception:
        inv_freq = (
            1.0 / (THETA ** (np.arange(HALF, dtype=np.float32) * 2.0 / HD))
        ).astype(np.float32)
        ang = np.arange(T, dtype=np.float32)[:, None] * inv_freq[None, :]
        cos = np.cos(ang).astype(np.float32)
        sin = np.sin(ang).astype(np.float32)
    return cos, sin


def _make_const_inputs(q_ln_w, q_ln_b, k_ln_w, k_ln_b):
    cos, sin = _rope_tables()  # [T, HALF] f32
    ropeP = np.zeros((P, NT, 2, HALF), np.float32)
    ropeP[:, :, 0] = cos.reshape(NT, P, HALF).transpose(1, 0, 2)
    ropeP[:, :, 1] = sin.reshape(NT, P, HALF).transpose(1, 0, 2)

    lnwb = np.zeros((P, 5, 2, HD), np.float32)
    for h in range(5):
        qk = 0 if h < QH else 1
        lnwb[:, h, 0] = np.asarray(q_ln_w if qk == 0 else k_ln_w, np.float32)[None, :]
        lnwb[:, h, 1] = np.asarray(q_ln_b if qk == 0 else k_ln_b, np.float32)[None, :]

    f = np.arange(QSPAN)[None, None, :]
    r = np.arange(4)[None, :, None]
    p = np.arange(P)[:, None, None]
    masks = (f >= 128 * r + p).astype(ml_dtypes.bfloat16)  # [P, 4, QSPAN]
    ident = np.eye(P, dtype=ml_dtypes.bfloat16)
    return ropeP, lnwb, masks, ident


def _split_fp8(a):
    """Exact-ish hi/lo e4m3 split of a float array: a ~= hi + lo with
    |a - hi - lo| <~ 2^-8 |a| (lo-subnormal dust only)."""
    f = np.asarray(a, np.float32)
    hi = f.astype(ml_dtypes.float8_e4m3)
    lo = (f - hi.astype(np.float32)).astype(ml_dtypes.float8_e4m3)
    return np.ascontiguousarray(hi), np.ascontiguousarray(lo)


# phase-3 lhsT rows are ordered (h, r, d) = head-of-rank h, rank r; the ao
# feature order is (global head g = 4r+h, d). Permute woT rows to match.
_WOT_PERM = np.empty(NH * HD, np.int64)
for _h in range(QH):
    for _r in range(NCORES):
        _j = (_h * NCORES + _r) * HD
        _g = (4 * _r + _h) * HD
        _WOT_PERM[_j : _j + HD] = np.arange(_g, _g + HD)


def make_in_maps(inputs, apply_lnwb=False):
    x = np.asarray(inputs["x"], dtype=ml_dtypes.bfloat16)
    wqkv = np.asarray(inputs["wqkv"], dtype=ml_dtypes.bfloat16)
    wo = np.asarray(inputs["wo"], dtype=ml_dtypes.bfloat16)
    q_ln_w = np.asarray(inputs["q_ln_w"], np.float32)
    q_ln_b = np.asarray(inputs["q_ln_b"], np.float32)
    k_ln_w = np.asarray(inputs["k_ln_w"], np.float32)
    k_ln_b = np.asarray(inputs["k_ln_b"], np.float32)

    ropeP, lnwb, masks, ident = _make_const_inputs(q_ln_w, q_ln_b, k_ln_w, k_ln_b)
    xT = np.ascontiguousarray(x.T)
    xTh, xTl = _split_fp8(xT)

    in_maps = []
    for c in range(NCORES):
        qrows = wqkv[c * QH * HD : (c + 1) * QH * HD]
        krows = wqkv[NH * HD + c * HD : NH * HD + (c + 1) * HD]
        vrows = wqkv[(NH + NKV) * HD + c * HD : (NH + NKV) * HD + (c + 1) * HD]
        wqkvT_c = np.concatenate([qrows, krows, vrows], axis=0).T
        wqkvTh_c, wqkvTl_c = _split_fp8(
            wqkvT_c.astype(np.float32) * WSCALE
        )
        woT_c = wo[c * OUTC : (c + 1) * OUTC, :].T[_WOT_PERM, :]
        woTh_c, woTl_c = _split_fp8(woT_c.astype(np.float32) * WSCALE)
        im = {
            "xTh": xTh,
            "xTl": xTl,
            "wqkvTh": wqkvTh_c,
            "wqkvTl": wqkvTl_c,
            "woTh": woTh_c,
            "woTl": woTl_c,
            "ropeP": ropeP,
            "masks": masks,
            "ident": ident,
        }
        if apply_lnwb:
            im["lnwb"] = lnwb
        in_maps.append(im)
    return in_maps


def kernel(**inputs):
    # the LN weight/bias application is elided when they are the identity
    # (which they are for this problem's inputs); fall back to the general
    # program otherwise
    apply_lnwb = not (
        np.all(np.asarray(inputs["q_ln_w"]) == 1.0)
        and np.all(np.asarray(inputs["k_ln_w"]) == 1.0)
        and np.all(np.asarray(inputs["q_ln_b"]) == 0.0)
        and np.all(np.asarray(inputs["k_ln_b"]) == 0.0)
    )
    nc = get_program(apply_lnwb=apply_lnwb)
    in_maps = make_in_maps(inputs, apply_lnwb=apply_lnwb)
    res = run_bass_kernel_spmd(nc, in_maps, list(range(NCORES)))
    outT_full = np.concatenate(
        [np.asarray(res.results[c]["outT"]) for c in range(NCORES)], axis=0
    )
    return np.ascontiguousarray(outT_full.T).astype(ml_dtypes.bfloat16)


if __name__ == "__main__":
    nc = get_program()
    print("program built ok")

